# revision 1
# baseline (speedup 1.0000x reference)
"""TRN2 Bass kernel v2 for nn_NodeEmbedding (3-relation GraphConv + PReLU).

y = PReLU( sum_r (D_in^-1/2 A_r D_out^-1/2 x) W_r + b_r )

Design (per core, 12500 dst nodes = 98 tiles of 128):
  - Edges partitioned by dst-owner core; per (relation, phase=src%4,
    tile-group) the needed x rows are fetched with one dma_gather
    (int16 idx = src//4 into a [25000, 512] fp16 view of x).
  - Routing/weighting/aggregation on the PE: for each 128-slot block and
    each dst tile it touches, matmul(lhsT=G_block[128 slots x 128 f],
    rhs=Wsel[128 slots x 128 dst]) accumulates aggT[f, dst] in PSUM.
    Wsel holds w_e = ns[src]*nd[dst] at (slot, dst%128), zeros elsewhere
    (host-precomputed fp16).
  - Phase B inline per tile group: aggT (PSUM) -> SBUF fp16, then
    y = sum_r aggT_r^T W_r + b via PE, PReLU on DVE, sequential store.
  No dma_scatter_add, no agg DRAM round-trip, no PE transposes.
"""

import numpy as np
from contextlib import ExitStack

P = 128
F = 128
N_CORES = 8

NPH = 4  # src phases (int16 gather idx = src//4 < 25000)


class Cfg:
    def __init__(self, n_nodes=100000, tg_tiles=8):
        self.N = n_nodes
        self.SHARD = n_nodes // N_CORES
        self.TILES = (self.SHARD + P - 1) // P
        self.NQ = n_nodes // NPH  # quad rows
        self.TGT = tg_tiles
        self.NGRP = (self.TILES + tg_tiles - 1) // tg_tiles


def _schedule(cfg, inputs):
    """Host preprocessing (index-side only).

    Returns (profile, percore) where
      profile[(r, ph)] = dict(nstar=[NGRP], pieces=[NGRP list of (block, tl)],
                              idxcols, wselcols)
      percore[c][(r, ph)] = dict(idx=[128, idxcols] int16,
                                 wsel=[128, wselcols] fp16)
    tl = tile index local to the group (0..TGT-1).
    """
    N, SHARD, TILES, TGT, NGRP = cfg.N, cfg.SHARD, cfg.TILES, cfg.TGT, cfg.NGRP
    ns, nd = [], []
    for r in range(3):
        src = np.asarray(inputs[f"src{r}"]).astype(np.int64)
        dst = np.asarray(inputs[f"dst{r}"]).astype(np.int64)
        deg_out = np.bincount(src, minlength=N)
        deg_in = np.bincount(dst, minlength=N)
        ns.append((1.0 / np.sqrt(np.maximum(deg_out, 1))).astype(np.float32))
        nd.append((1.0 / np.sqrt(np.maximum(deg_in, 1))).astype(np.float32))

    # per (core, r): sorted edge arrays and per-(ph, tile) counts
    edata = {}
    cnt = np.zeros((N_CORES, 3, NPH, TILES), np.int64)
    for r in range(3):
        src = np.asarray(inputs[f"src{r}"]).astype(np.int64)
        dst = np.asarray(inputs[f"dst{r}"]).astype(np.int64)
        w_all = ns[r][src] * nd[r][dst]
        core = dst // SHARD
        for c in range(N_CORES):
            m = core == c
            s, d, w = src[m], dst[m] - c * SHARD, w_all[m]
            ph, q = s % NPH, s // NPH
            tile = d // P
            order = np.lexsort((tile, ph))  # primary ph, secondary tile
            edata[c, r] = (q[order], tile[order], (d % P)[order], w[order],
                           ph[order])
            np.add.at(cnt[c, r], (ph[order], tile[order]), 1)

    # common num_idxs per (r, ph, tg), 16-aligned
    grp_cnt = cnt.reshape(N_CORES, 3, NPH, NGRP, -1).sum(-1) \
        if TILES % TGT == 0 else None
    if grp_cnt is None:
        pad = (-TILES) % TGT
        cp = np.pad(cnt, ((0, 0), (0, 0), (0, 0), (0, pad)))
        grp_cnt = cp.reshape(N_CORES, 3, NPH, NGRP, TGT).sum(-1)
    nstar = grp_cnt.max(0)  # [3, NPH, NGRP]
    nstar = ((nstar + 15) // 16) * 16

    profile = {}
    percore = [dict() for _ in range(N_CORES)]
    for r in range(3):
        for ph in range(NPH):
            nst = nstar[r, ph]  # [NGRP]
            goff = np.concatenate([[0], np.cumsum(nst)])  # slot offset per tg
            total = int(goff[-1])
            # per-core slot assignment
            slot_of = []  # per core: slot index per edge (in sorted order)
            for c in range(N_CORES):
                q, tile, dstoff, w, phv = edata[c, r]
                m = phv == ph
                tl_tile = tile[m]
                tg = tl_tile // TGT
                # rank within (tg) segment: edges sorted by tile so by tg too
                seg_start = np.searchsorted(tg, np.arange(NGRP), side="left")
                ranks = np.arange(tg.size) - seg_start[tg]
                slot = goff[tg] + ranks
                slot_of.append((m, slot))
            # union pieces per tg: (block within tg, tl)
            pieces = [set() for _ in range(NGRP)]
            for c in range(N_CORES):
                ccnt = cnt[c, r, ph]  # [TILES]
                for g in range(NGRP):
                    t0, t1 = g * TGT, min((g + 1) * TGT, TILES)
                    cc = ccnt[t0:t1]
                    ends = np.cumsum(cc)
                    starts = ends - cc
                    for tl in range(t1 - t0):
                        if cc[tl] == 0:
                            continue
                        b0 = starts[tl] // P
                        b1 = (ends[tl] - 1) // P
                        for b in range(b0, b1 + 1):
                            pieces[g].add((int(b), int(tl)))
            pieces = [sorted(ps) for ps in pieces]
            piece_base = np.concatenate(
                [[0], np.cumsum([len(ps) for ps in pieces])])
            npieces = int(piece_base[-1])
            # per-core arrays
            pidx_lookup = [
                {bt: i for i, bt in enumerate(ps)} for ps in pieces]
            for c in range(N_CORES):
                q, tile, dstoff, w, phv = edata[c, r]
                m, slot = slot_of[c]
                qm, tm, dm, wm = q[m], tile[m], dstoff[m], w[m]
                tgm = tm // TGT
                tlm = tm - tgm * TGT
                lslot = slot - goff[tgm]
                blk = lslot // P
                # slot numbering restarts per gather instruction (per tg)
                row = lslot % P
                maxb = max((b for ps in pieces for b, _ in ps), default=0) + 1
                parr = np.full((NGRP, maxb, TGT), -1, np.int64)
                for g_, ps in enumerate(pieces):
                    for i_, (b_, tl_) in enumerate(ps):
                        parr[g_, b_, tl_] = piece_base[g_] + i_
                pcol = parr[tgm, blk, tlm]
                assert (pcol >= 0).all()
                idx_flat = np.zeros(total, np.int16)
                idx_flat[slot] = qm.astype(np.int16)
                ar = np.arange(total)
                idx_wrap = np.zeros((16, total // 16), np.int16)
                idx_wrap[ar % 16, ar // 16] = idx_flat
                wsel = np.zeros((P, npieces * P), np.float16)
                wsel[row, pcol * P + dm] = wm.astype(np.float16)
                percore[c][r, ph] = dict(
                    idx=np.tile(idx_wrap, (8, 1)), wsel=wsel)
            profile[r, ph] = dict(
                nstar=[int(v) for v in nst], pieces=pieces,
                idxcols=total // 16, wselcols=max(npieces * P, P))
    return profile, percore


def _build_bass(cfg, profile):
    import concourse.bass as bass
    import concourse.bacc as bacc
    import concourse.tile as tile
    import concourse.mybir as mybir

    TILES, SHARD, TGT, NGRP = cfg.TILES, cfg.SHARD, cfg.TGT, cfg.NGRP
    dt = mybir.dt

    nc = bacc.Bacc("TRN2", target_bir_lowering=False, debug=False,
                   num_devices=N_CORES)

    x4h = nc.dram_tensor("x4h", [cfg.NQ, NPH * F], dt.float16,
                         kind="ExternalInput")
    idx_t, wsel_t = {}, {}
    for r in range(3):
        for ph in range(NPH):
            pr = profile[r, ph]
            idx_t[r, ph] = nc.dram_tensor(
                f"idx{r}_{ph}", [P, pr["idxcols"]], dt.int16,
                kind="ExternalInput")
            wsel_t[r, ph] = nc.dram_tensor(
                f"wsel{r}_{ph}", [P, pr["wselcols"]], dt.float16,
                kind="ExternalInput")
    W_t = [nc.dram_tensor(f"W{r}", [F, F], dt.float16, kind="ExternalInput")
           for r in range(3)]
    b_t = nc.dram_tensor("bvec", [1, 512], dt.float16, kind="ExternalInput")
    pa_t = nc.dram_tensor("prelu_a", [1], dt.float32, kind="ExternalInput")
    y_t = nc.dram_tensor("y", [TILES * P, F], dt.float32,
                         kind="ExternalOutput")

    with tile.TileContext(nc) as tc:
        with ExitStack() as ctx:
            cpool = ctx.enter_context(tc.tile_pool(name="const", bufs=1))
            ipool = ctx.enter_context(tc.tile_pool(name="idx", bufs=1))
            gpool = ctx.enter_context(tc.tile_pool(name="g", bufs=3))
            wpool = ctx.enter_context(tc.tile_pool(name="wsel", bufs=3))
            apool = ctx.enter_context(tc.tile_pool(name="aggsb", bufs=2))
            ypool = ctx.enter_context(tc.tile_pool(name="y", bufs=2))

            # constants
            W_sb = []
            for r in range(3):
                w_ = cpool.tile([F, F], dt.float16, tag=f"W{r}")
                nc.sync.dma_start(w_[:], W_t[r][:, :])
                W_sb.append(w_)
            b_sb = cpool.tile([1, 512], dt.float16)
            nc.sync.dma_start(b_sb[:], b_t[:, :])
            ones1 = cpool.tile([1, P], dt.float16)
            nc.vector.memset(ones1[:], 1.0)
            ones1f = cpool.tile([1, P], dt.float32)
            nc.vector.memset(ones1f[:], 1.0)
            zvec = cpool.tile([P, 512], dt.float16)
            nc.vector.memset(zvec[:], 0.0)
            pa_sb = cpool.tile([1, 1], dt.float32)
            nc.sync.dma_start(pa_sb[:], pa_t[None, :])
            am1 = cpool.tile([P, 1], dt.float32)
            with tc.tile_pool(name="ppsum", bufs=1, space="PSUM") as ppool:
                pa_ps = ppool.tile([P, 1], dt.float32, space="PSUM")
                nc.tensor.matmul(pa_ps[:], lhsT=ones1f[:], rhs=pa_sb[:],
                                 start=True, stop=True)
                nc.vector.tensor_scalar_add(am1[:], pa_ps[:], -1.0)

            pagg = ctx.enter_context(
                tc.tile_pool(name="pagg", bufs=1, space="PSUM"))
            py_pool = ctx.enter_context(
                tc.tile_pool(name="py", bufs=2, space="PSUM"))

            # preload all idx arrays
            idx_sb = {}
            for r in range(3):
                for ph in range(NPH):
                    t = ipool.tile([P, profile[r, ph]["idxcols"]], dt.int16,
                                   tag=f"i{r}_{ph}")
                    nc.sync.dma_start(t[:], idx_t[r, ph][:, :])
                    idx_sb[r, ph] = t

            # zero G pool slots once: pieces may read rows no gather wrote
            # (per-core counts < common max); raw SBUF can be NaN patterns
            # and PE NaN*0 = NaN.
            maxnb = 0
            for r in range(3):
                for ph in range(NPH):
                    for n_ in profile[r, ph]["nstar"]:
                        maxnb = max(maxnb, (n_ + P - 1) // P)
            for _ in range(3):
                gz = gpool.tile([P, maxnb, F], dt.float16, tag="G")
                nc.vector.memset(gz[:], 0.0)

            for g in range(NGRP):
                t0 = g * TGT
                t1 = min(t0 + TGT, TILES)
                ntl = t1 - t0
                nbank = (ntl + 3) // 4
                aggps = {}
                for r in range(3):
                    for bk in range(nbank):
                        ps = pagg.tile([P, 512], dt.float32, space="PSUM",
                                       tag=f"agg{r}_{bk}")
                        # zero the bank and set accumulate bits everywhere
                        nc.tensor.matmul(ps[:, :], lhsT=zvec[:, 0:P],
                                         rhs=zvec[:, 0:512], start=True,
                                         stop=False, skip_group_check=True)
                        aggps[r, bk] = ps
                for r in range(3):
                    for ph in range(NPH):
                        pr = profile[r, ph]
                        n = pr["nstar"][g]
                        if n == 0:
                            continue
                        pieces = pr["pieces"][g]
                        nb = (n + P - 1) // P
                        o16 = sum(pr["nstar"][:g]) // 16
                        po = sum(len(pr["pieces"][k]) for k in range(g))
                        G = gpool.tile([P, nb, F], dt.float16, tag="G")
                        nc.gpsimd.dma_gather(
                            out_ap=G[:, :, :],
                            in_ap=x4h[:, ph * F:(ph + 1) * F],
                            idxs_ap=idx_sb[r, ph][:, o16:o16 + n // 16],
                            num_idxs=n, num_idxs_reg=n, elem_size=F,
                            elem_step=NPH * F,
                            single_packet=(n <= 1024))
                        if pieces:
                            wse = wpool.tile([P, len(pieces) * P], dt.float16,
                                             tag="wsel")
                            nc.sync.dma_start(
                                wse[:],
                                wsel_t[r, ph][:, po * P:(po + len(pieces)) * P])
                            for k, (b, tl) in enumerate(pieces):
                                bk, c0 = tl // 4, (tl % 4) * P
                                nc.tensor.matmul(
                                    aggps[r, bk][:, c0:c0 + P],
                                    lhsT=G[:, b, :],
                                    rhs=wse[:, k * P:(k + 1) * P],
                                    start=False, stop=False,
                                    skip_group_check=True)
                # ---- phase B for this tile group ----
                aggsb = {}
                for r in range(3):
                    for bk in range(nbank):
                        asb = apool.tile([P, 512], dt.float16,
                                         tag=f"as{r}_{bk}")
                        nc.vector.tensor_copy(asb[:], aggps[r, bk][:, :])
                        aggsb[r, bk] = asb
                for bk in range(nbank):
                    yps = py_pool.tile([P, 512], dt.float32, space="PSUM",
                                       tag="yps")
                    nc.tensor.matmul(yps[:, :], lhsT=ones1[:], rhs=b_sb[:, :],
                                     start=True, stop=False,
                                     skip_group_check=True)
                    for tl4 in range(min(4, ntl - bk * 4)):
                        c0 = tl4 * P
                        for r in range(3):
                            nc.tensor.matmul(
                                yps[:, c0:c0 + P],
                                lhsT=aggsb[r, bk][:, c0:c0 + P],
                                rhs=W_sb[r][:, :],
                                start=False, stop=False,
                                skip_group_check=True)
                    neg = ypool.tile([P, 512], dt.float32, tag="neg")
                    nc.vector.tensor_scalar_min(neg[:], yps[:, :], 0.0)
                    ysb = ypool.tile([P, 512], dt.float32, tag="ysb")
                    nc.vector.scalar_tensor_tensor(
                        out=ysb[:], in0=neg[:], scalar=am1[:, :1],
                        in1=yps[:, :],
                        op0=mybir.AluOpType.mult,
                        op1=mybir.AluOpType.add)
                    for tl4 in range(min(4, ntl - bk * 4)):
                        t = t0 + bk * 4 + tl4
                        nc.sync.dma_start(
                            y_t[t * P:(t + 1) * P, :],
                            ysb[:, tl4 * P:(tl4 + 1) * P])

    nc.compile()
    return nc


_NC_CACHE = {}


def _profile_key(profile):
    import hashlib
    h = hashlib.sha256()
    for k in sorted(profile):
        pr = profile[k]
        h.update(repr((k, pr["nstar"], pr["pieces"])).encode())
    return h.hexdigest()


def _run(cfg, inputs, trace=False, trace_kwargs=None):
    from concourse.bass_utils import run_bass_kernel_spmd

    x = np.ascontiguousarray(np.asarray(inputs["x"], dtype=np.float32))
    profile, percore = _schedule(cfg, inputs)
    key = (cfg.N, cfg.TGT, _profile_key(profile))
    nc = _NC_CACHE.get(key)
    if nc is None:
        nc = _build_bass(cfg, profile)
        _NC_CACHE.clear()
        _NC_CACHE[key] = nc

    x4h = x.astype(np.float16).reshape(cfg.NQ, NPH * F)
    bsum = (np.asarray(inputs["b0"]) + np.asarray(inputs["b1"])
            + np.asarray(inputs["b2"])).astype(np.float16)
    bvec = np.tile(bsum, 4)[None, :]
    in_maps = []
    for c in range(N_CORES):
        m = {"x4h": x4h, "bvec": bvec,
             "prelu_a": np.asarray(inputs["prelu_a"], dtype=np.float32)}
        for r in range(3):
            m[f"W{r}"] = np.asarray(inputs[f"W{r}"],
                                    dtype=np.float32).astype(np.float16)
            for ph in range(NPH):
                m[f"idx{r}_{ph}"] = percore[c][r, ph]["idx"]
                w = percore[c][r, ph]["wsel"]
                want = profile[r, ph]["wselcols"]
                if w.shape[1] < want:
                    w = np.pad(w, ((0, 0), (0, want - w.shape[1])))
                m[f"wsel{r}_{ph}"] = w
        in_maps.append(m)

    res = run_bass_kernel_spmd(nc, in_maps, core_ids=list(range(N_CORES)),
                               trace=trace, **(trace_kwargs or {}))
    y = np.concatenate(
        [res.results[c]["y"][:cfg.SHARD] for c in range(N_CORES)], axis=0)
    return y, res


def kernel(**inputs) -> np.ndarray:
    cfg = Cfg()
    y, _ = _run(cfg, inputs)
    return y.astype(np.float32)


if __name__ == "__main__":
    pass

